# revision 26
# baseline (speedup 1.0000x reference)
"""CLIP encoder layer on 8 Trainium2 NeuronCores, data-parallel over batch.

Full (unsharded) inputs -> full output.  Each core runs the whole layer for
one batch element (B == 8 == n_cores), so there are no collectives.

All matmul operands in bf16 (rel err ~2e-3 vs the 2e-2 gate), PSUM
accumulation fp32.  Layout strategy per core:
  - LayerNorm token-major (bn_stats), normalized tile emitted bf16 and
    transposed per 128x128 block on the PE into feature-major h1T/h2T.
  - Q/K produced feature-major [D, L] (stationary = retiled weight blocks),
    bias folded into the ACT eviction.
  - V, out-proj and fc2 are produced TOKEN-major directly (stationary =
    activation blocks, moving = weight rows), which kills the transpose
    passes and writes V straight into the token-major V65 layout (ones
    column per head folds the softmax denominator into the attnV matmul).
  - Scores are key-major S^T so softmax reduces over the PSUM partition
    dim; exp is one ACT op with scale/mask folded in; O^T = V65^T @ expS;
    the per-head 1/denom is a single bf16 DVE reciprocal broadcast via a
    DRAM partition-broadcast DMA (PE ones-broadcast for the tail head).
  - Residual stream x1 stays SBUF-resident (no DRAM round trip); fc2
    output accumulates token-major and is evicted fused with x1 + b2.
  - LN affine and LN bias are folded into the consuming weights on host.
"""
from contextlib import ExitStack

import numpy as np
import ml_dtypes

import concourse.bacc as bacc
import concourse.tile as tile
from concourse import mybir
from concourse.masks import make_identity

B, L, D = 8, 1024, 1024
H, HD, FF = 16, 64, 4096
EPS = 1e-5
P = 128
NCORES = 8
TC = L // P      # 8 token tiles
FC = D // P      # 8 feature tiles
MC = FF // P     # 32 ff tiles

f32 = mybir.dt.float32
bf16 = mybir.dt.bfloat16
i32 = mybir.dt.int32
AF = mybir.ActivationFunctionType
ALU = mybir.AluOpType
NPBF = ml_dtypes.bfloat16


def build_nc(replicas=1):
    nc = bacc.Bacc(None, dynamic_dma_scratch_size=8192)

    t = {}
    t["xd"] = nc.dram_tensor("x", [L, D], f32, kind="ExternalInput")
    t["maskd"] = nc.dram_tensor("mask", [L], i32, kind="ExternalInput")
    t["wqr"] = nc.dram_tensor("wqr", [FC, P, FC, P], bf16, kind="ExternalInput")
    t["wkr"] = nc.dram_tensor("wkr", [FC, P, FC, P], bf16, kind="ExternalInput")
    t["wvr"] = nc.dram_tensor("wvr", [FC, P, D], bf16, kind="ExternalInput")
    t["wor"] = nc.dram_tensor("wor", [FC, P, D], bf16, kind="ExternalInput")
    t["w1r"] = nc.dram_tensor("w1r", [MC, P, FC, P], bf16, kind="ExternalInput")
    t["w2r"] = nc.dram_tensor("w2r", [MC, P, D], bf16, kind="ExternalInput")
    for nm, n in (("bq", D), ("bk", D), ("bvd", D), ("bod", D), ("b1", FF),
                  ("b2d", D)):
        t[nm] = nc.dram_tensor(nm, [n], f32, kind="ExternalInput")
    t["yd"] = nc.dram_tensor("y", [L, D], f32, kind="ExternalOutput")

    with tile.TileContext(nc) as tc:
        with ExitStack() as ctx:
            pools = _make_pools(tc, ctx)
            consts = _emit_consts(nc, pools, t)
            x_src = t["xd"]
            for r in range(replicas):
                last = r == replicas - 1
                y_dst = t["yd"] if last else nc.dram_tensor(f"ychain{r}", [L, D], f32)
                rec_scr = nc.dram_tensor(f"rec_scratch{r}", [H, L], bf16)
                _emit_layer(nc, pools, consts, t, x_src, y_dst, rec_scr)
                x_src = y_dst
    nc.compile()
    return nc


def _make_pools(tc, ctx):
    p = {}
    p["big"] = ctx.enter_context(tc.tile_pool(name="big", bufs=3))
    p["gp"] = ctx.enter_context(tc.tile_pool(name="gp", bufs=1))
    p["v65p"] = ctx.enter_context(tc.tile_pool(name="v65p", bufs=1))
    p["x1p"] = ctx.enter_context(tc.tile_pool(name="x1p", bufs=1))
    p["expp"] = ctx.enter_context(tc.tile_pool(name="expp", bufs=5))
    p["nst"] = ctx.enter_context(tc.tile_pool(name="nst", bufs=4))
    p["wp"] = ctx.enter_context(tc.tile_pool(name="wp", bufs=4))
    p["w2p"] = ctx.enter_context(tc.tile_pool(name="w2p", bufs=9))
    p["otp"] = ctx.enter_context(tc.tile_pool(name="otp", bufs=2))
    p["bcp"] = ctx.enter_context(tc.tile_pool(name="bcp", bufs=3))
    p["smal"] = ctx.enter_context(tc.tile_pool(name="smal", bufs=1))
    p["stat"] = ctx.enter_context(tc.tile_pool(name="stat", bufs=2))
    p["mmS"] = ctx.enter_context(tc.tile_pool(name="mmS", bufs=2, space="PSUM"))
    p["mmO"] = ctx.enter_context(tc.tile_pool(name="mmO", bufs=2, space="PSUM"))
    return p


def _emit_consts(nc, p, t):
    smal = p["smal"]
    c = {}

    ident_st = smal.tile([P, P], f32, tag="ident_st")
    make_identity(nc, ident_st[:])
    ident = smal.tile([P, P], bf16, tag="ident")
    nc.vector.tensor_copy(out=ident[:], in_=ident_st[:])
    c["ident"] = ident

    ones_r = smal.tile([1, 64], bf16, tag="ones_r")
    nc.vector.memset(ones_r[:], 1.0)
    c["ones_r"] = ones_r

    def load_vec(name, n):
        tl = smal.tile([P, n // P], f32, tag=name + "t", name=name + "t")
        nc.gpsimd.dma_start(out=tl[:], in_=t[name][:].rearrange("(c p) -> p c", p=P))
        return tl

    c["bq"] = load_vec("bq", D)
    c["bk"] = load_vec("bk", D)
    c["b1"] = load_vec("b1", FF)

    # free-dim broadcast bias tiles [P, D] (token-major evictions)
    import concourse.bass as bass
    for nm, key in (("bvd", "bvbc"), ("bod", "bobc"), ("b2d", "b2bc")):
        bt32 = smal.tile([P, D], f32, tag="bbc32", name=key + "32")
        src = bass.AP(tensor=t[nm], offset=0, ap=[[0, P], [1, D]])
        nc.sync.dma_start(out=bt32[:], in_=src)
        bt = smal.tile([P, D], bf16, tag=key, name=key)
        nc.vector.tensor_copy(out=bt[:], in_=bt32[:])
        c[key] = bt

    epst = smal.tile([P, 1], f32, tag="epst")
    nc.vector.memset(epst[:], EPS)
    c["eps"] = epst

    # additive key mask: (m - 1) * 1e30  ->  0 or -1e30
    mi = smal.tile([P, TC], i32, tag="mi")
    nc.gpsimd.dma_start(out=mi[:], in_=t["maskd"][:].rearrange("(t p) -> p t", p=P))
    mf = smal.tile([P, TC], f32, tag="mf")
    nc.vector.tensor_copy(out=mf[:], in_=mi[:])
    fmask = smal.tile([P, TC], f32, tag="fmask")
    nc.vector.tensor_scalar(out=fmask[:], in0=mf[:], scalar1=1.0, scalar2=1e30,
                            op0=ALU.subtract, op1=ALU.mult)
    c["fmask"] = fmask

    ones_col = smal.tile([P, TC, H], bf16, tag="ones_col")
    nc.vector.memset(ones_col[:], 1.0)
    c["ones_col"] = ones_col
    return c


def _emit_layer(nc, p, c, t, xd, yd, recd):
    big, gp, v65p, x1p = p["big"], p["gp"], p["v65p"], p["x1p"]
    expp, nst, wp, otp, bcp = p["expp"], p["nst"], p["wp"], p["otp"], p["bcp"]
    stat, mmS, mmO, smal = p["stat"], p["mmS"], p["mmO"], p["smal"]
    ident, fmask, ones_r = c["ident"], c["fmask"], c["ones_r"]
    import concourse.bass as bass

    # V65: token-major V with a ones column per head
    v65 = v65p.tile([P, TC, H * 65], bf16, tag="v65", name="v65")
    v65_ones = v65[:, :, :].rearrange("p t (h c) -> p t h c", c=65)[:, :, :, 64]
    nc.vector.tensor_copy(out=v65_ones, in_=c["ones_col"][:])

    # resident residual stream (bf16: one rounding of the residual costs
    # ~1e-3 rel err against the 2e-2 gate and halves the SBUF footprint)
    x1 = x1p.tile([P, TC, L], bf16, tag="x1", name="x1")

    def layernorm_tile(x_tc):
        """token-major [128, D] f32 -> normalized bf16 tile."""
        st = stat.tile([P, 2, nc.vector.BN_STATS_DIM], f32, tag="bnst", name="st")
        xg = x_tc.rearrange("p (s f) -> p s f", s=2)
        for s in range(2):
            nc.vector.bn_stats(out=st[:, s, :], in_=xg[:, s, :])
        mv = stat.tile([P, nc.vector.BN_AGGR_DIM], f32, tag="bnmv", name="mv")
        nc.vector.bn_aggr(out=mv[:], in_=st[:])
        sd = stat.tile([P, 1], f32, tag="bnsd", name="sd")
        nc.scalar.activation(sd[:], mv[:, 1:2], AF.Sqrt, bias=c["eps"][:], scale=1.0)
        r0 = stat.tile([P, 1], f32, tag="bnr0", name="r0")
        nc.vector.reciprocal(out=r0[:], in_=sd[:])
        t1 = stat.tile([P, 1], f32, tag="bnt1", name="t1")
        nc.vector.tensor_mul(t1[:], sd[:], r0[:])
        nc.vector.tensor_scalar(out=t1[:], in0=t1[:], scalar1=-1.0, scalar2=2.0,
                                op0=ALU.mult, op1=ALU.add)
        rstd = stat.tile([P, 1], f32, tag="bnrstd", name="rstd")
        nc.vector.tensor_mul(rstd[:], r0[:], t1[:])
        nmu = stat.tile([P, 1], f32, tag="bnnmu", name="nmu")
        nc.vector.tensor_scalar(out=nmu[:], in0=mv[:, 0:1], scalar1=rstd[:],
                                scalar2=-1.0, op0=ALU.mult, op1=ALU.mult)
        n_tc = nst.tile([P, D], bf16, tag="nstage", name="n_tc")
        nc.scalar.activation(n_tc[:], x_tc, AF.Identity, bias=nmu[:],
                             scale=rstd[:])
        return n_tc

    def transpose_to(dstT, n_tc, tt):
        """Transpose [128, D] token-major bf16 tile into the tt-th token
        column of feature-major dstT via one PSUM bank + one ACT evict."""
        tp = mmO.tile([P, D], bf16, tag="mmO", name="tp")
        for cc in range(FC):
            nc.tensor.transpose(tp[:, cc * P:(cc + 1) * P],
                                n_tc[:, cc * P:(cc + 1) * P], ident[:])
        nc.scalar.activation(
            dstT[:, :, tt * P:(tt + 1) * P],
            tp[:].rearrange("p (j q) -> p j q", j=FC), AF.Copy)

    # ---------------- LN1 -> h1T (feature-major, bf16) ----------------
    h1T = big.tile([P, FC, L], bf16, tag="big", name="h1T")
    prev_ln = None
    for tt in range(TC):
        x_tc = nst.tile([P, D], f32, tag="xstage", name="x_tc")
        nc.sync.dma_start(out=x_tc[:], in_=xd[tt * P:(tt + 1) * P, :])
        if prev_ln is not None:
            transpose_to(h1T, *prev_ln)
        n_tc = layernorm_tile(x_tc[:])
        prev_ln = (n_tc, tt)
    transpose_to(h1T, *prev_ln)

    # ------------- V projection, token-major -> V65 ------------------
    for tg in range(2):
        ps_v = [(mmS if i < 2 else mmO).tile([P, L], f32, tag="mmS" if i < 2 else "mmO",
                                             name=f"psv{tg}_{i}") for i in range(4)]
        for k in range(FC):
            wvt = wp.tile([P, D], bf16, tag="w", name="wvt")
            nc.sync.dma_start(out=wvt[:], in_=t["wvr"][k])
            for i in range(4):
                tt = tg * 4 + i
                for half in range(2):
                    nc.tensor.matmul(
                        ps_v[i][:, half * 512:(half + 1) * 512],
                        h1T[:, k, tt * P:(tt + 1) * P],
                        wvt[:, half * 512:(half + 1) * 512],
                        start=(k == 0), stop=(k == FC - 1))
        for i in range(4):
            tt = tg * 4 + i
            for half in range(2):
                dst = v65[:, tt, :].rearrange(
                    "p (h q) -> p h q", q=65)[:, half * 8:(half + 1) * 8, :64]
                nc.vector.tensor_add(
                    dst,
                    ps_v[i][:, half * 512:(half + 1) * 512].rearrange(
                        "p (h q) -> p h q", q=64),
                    c["bvbc"][:, half * 512:(half + 1) * 512].rearrange(
                        "p (h q) -> p h q", q=64))

    # ---------------- Q/K projections (feature-major) ----------------
    qT = big.tile([P, FC, L], bf16, tag="big", name="qT")
    kT = big.tile([P, FC, L], bf16, tag="big", name="kT")
    for dst, wsrc, bias in ((qT, t["wqr"], c["bq"]), (kT, t["wkr"], c["bk"])):
        for fc in range(FC):
            wt = wp.tile([P, FC, P], bf16, tag="w", name="wqk")
            nc.sync.dma_start(out=wt[:], in_=wsrc[fc])
            ps = mmS.tile([P, L], f32, tag="mmS", name="psqk")
            for half in range(2):
                for kt in range(FC):
                    nc.tensor.matmul(
                        ps[:, half * 512:(half + 1) * 512], wt[:, kt, :],
                        h1T[:, kt, half * 512:(half + 1) * 512],
                        start=(kt == 0), stop=(kt == FC - 1))
            nc.scalar.activation(dst[:, fc, :], ps[:], AF.Identity,
                                 bias=bias[:, fc:fc + 1], scale=1.0)

    # ---------------- attention ----------------
    attnT = big.tile([P, FC, L], bf16, tag="big", name="attnT")

    def head_epilogue(h, ot_ps, fast=False):
        p0 = (h % 2) * 64
        hc = h // 2
        ots = otp.tile([64, L], f32, tag="otdiv", name="ots")
        nc.vector.tensor_copy(out=ots[:], in_=ot_ps[0:64, :])
        bc = bcp.tile([64, L], bf16, tag="bc", name="bc")
        recr = bcp.tile([1, L], bf16, tag="bcs", name="recr")
        with nc.allow_low_precision(reason="softmax 1/denom in bf16 is ~4e-3"):
            nc.vector.reciprocal(out=recr[:], in_=ot_ps[64:65, :])
        if fast:
            bc_ps = mmS.tile([64, L], f32, tag="mmS", name="bc_ps")
            for half in range(2):
                nc.tensor.matmul(bc_ps[:, half * 512:(half + 1) * 512],
                                 ones_r[:], recr[:, half * 512:(half + 1) * 512],
                                 start=True, stop=True)
            nc.scalar.activation(bc[:], bc_ps[:], AF.Copy)
        else:
            nc.sync.dma_start(out=recd[h:h + 1, :], in_=recr[:])
            bcast_src = bass.AP(tensor=recd, offset=h * L,
                                ap=[[0, 64], [1, L]])
            nc.sync.dma_start(out=bc[:], in_=bcast_src)
        if p0 == 0:
            nc.vector.tensor_mul(attnT[0:64, hc, :], ots[:], bc[:])
        else:
            ots2 = otp.tile([64, L], bf16, tag="otdiv", name="ots2")
            nc.vector.tensor_mul(ots2[:], ots[:], bc[:])
            nc.gpsimd.dma_start(out=attnT[64:128, hc, :], in_=ots2[:])

    head_order = [h for h in range(H) if h % 2 == 1] + \
                 [h for h in range(H) if h % 2 == 0]
    n_units = H * TC
    ess = {}

    def score_unit(u):
        h = head_order[u // TC]
        kt = u % TC
        p0 = (h % 2) * 64
        hc = h // 2
        st_ps = mmS.tile([P, L], f32, tag="mmS", name=f"st_ps{h}_{kt}")
        for half in range(2):
            nc.tensor.matmul(
                st_ps[:, half * 512:(half + 1) * 512],
                kT[p0:p0 + 64, hc, kt * P:(kt + 1) * P],
                qT[p0:p0 + 64, hc, half * 512:(half + 1) * 512],
                start=True, stop=True)
        es = expp.tile([P, L], bf16, tag="expS", name="es")
        nc.scalar.activation(es[:], st_ps[:], AF.Exp,
                             bias=fmask[:, kt:kt + 1], scale=0.125)
        ess[u] = es

    score_unit(0)
    score_unit(1)
    score_unit(2)
    prev_ot = None
    prev_h = None
    for hi, h in enumerate(head_order):
        if prev_ot is not None:
            head_epilogue(prev_h, prev_ot)
        ot_ps = mmO.tile([65, L], f32, tag="mmO", name=f"ot_ps{h}")
        for kt in range(TC):
            u = hi * TC + kt
            if u + 3 < n_units:
                score_unit(u + 3)
            es = ess.pop(u)
            for half in range(2):
                nc.tensor.matmul(
                    ot_ps[:, half * 512:(half + 1) * 512],
                    v65[:, kt, h * 65:(h + 1) * 65],
                    es[:, half * 512:(half + 1) * 512],
                    start=(kt == 0), stop=(kt == TC - 1))
        prev_ot = ot_ps
        prev_h = h
    head_epilogue(prev_h, prev_ot, fast=True)

    # ------- out projection token-major + residual -> x1 (SBUF) -------
    for tg in range(2):
        xv = []
        for i in range(4):
            tt = tg * 4 + i
            x2 = nst.tile([P, D], f32, tag="xstage", name="x2")
            nc.sync.dma_start(out=x2[:], in_=xd[tt * P:(tt + 1) * P, :])
            nc.vector.tensor_add(x2[:], x2[:], c["bobc"][:])
            xv.append(x2)
        ps_z = [(mmS if i < 2 else mmO).tile([P, L], f32, tag="mmS" if i < 2 else "mmO",
                                             name=f"psz{tg}_{i}") for i in range(4)]
        for k in range(FC):
            wot = wp.tile([P, D], bf16, tag="w", name="wot")
            nc.sync.dma_start(out=wot[:], in_=t["wor"][k])
            for i in range(4):
                tt = tg * 4 + i
                for half in range(2):
                    nc.tensor.matmul(
                        ps_z[i][:, half * 512:(half + 1) * 512],
                        attnT[:, k, tt * P:(tt + 1) * P],
                        wot[:, half * 512:(half + 1) * 512],
                        start=(k == 0), stop=(k == FC - 1))
        for i in range(4):
            tt = tg * 4 + i
            for half in range(2):
                sl = slice(half * 512, (half + 1) * 512)
                nc.vector.tensor_add(x1[:, tt, sl], ps_z[i][:, sl], xv[i][:, sl])

    # -------- LN2 -> h2T; then x1 += b2 broadcast ---------------------
    h2T = big.tile([P, FC, L], bf16, tag="big", name="h2T")
    prev_ln = None
    for tt in range(TC):
        if prev_ln is not None:
            transpose_to(h2T, *prev_ln)
        n_tc = layernorm_tile(x1[:, tt, :])
        prev_ln = (n_tc, tt)
        nc.vector.tensor_add(x1[:, tt, :], x1[:, tt, :], c["b2bc"][:])
    transpose_to(h2T, *prev_ln)

    # ---------------- MLP in two token halves ----------------
    for half in range(2):
        hsl = slice(half * 512, (half + 1) * 512)
        g = gp.tile([P, MC, 512], bf16, tag="g", name=f"g{half}")
        for m in range(MC):
            wt = wp.tile([P, FC, P], bf16, tag="w", name="w1t")
            nc.sync.dma_start(out=wt[:], in_=t["w1r"][m])
            ps = mmS.tile([P, 512], f32, tag="mmS", name="psf1")
            for kt in range(FC):
                nc.tensor.matmul(
                    ps[:], wt[:, kt, :], h2T[:, kt, hsl],
                    start=(kt == 0), stop=(kt == FC - 1))
            nc.scalar.activation(g[:, m, :], ps[:], AF.Gelu_apprx_sigmoid,
                                 bias=c["b1"][:, m:m + 1], scale=1.0)
        ps_y = [(mmS if i < 2 else mmO).tile([P, L], f32, tag="mmS" if i < 2 else "mmO",
                                             name=f"psy{half}_{i}") for i in range(4)]
        # k blocked by 8 with the block's w2 rows resident, so each PSUM
        # bank gets 8 back-to-back matmuls before the PE moves on (fine
        # bank-cycling between accumulation groups makes HAM oscillate).
        w2p = p["w2p"]
        for kg in range(4):
            wts = []
            for k8 in range(8):
                wt = w2p.tile([P, D], bf16, tag="w2", name="w2t")
                nc.sync.dma_start(out=wt[:], in_=t["w2r"][kg * 8 + k8])
                wts.append(wt)
            for i in range(4):
                for dh in range(2):
                    for k8 in range(8):
                        nc.tensor.matmul(
                            ps_y[i][:, dh * 512:(dh + 1) * 512],
                            g[:, kg * 8 + k8, (i * P):(i + 1) * P],
                            wts[k8][:, dh * 512:(dh + 1) * 512],
                            start=(kg == 0 and k8 == 0),
                            stop=(kg == 3 and k8 == 7))
        for i in range(4):
            tb = half * 4 + i
            yt = nst.tile([P, D], f32, tag="xstage", name="yt")
            for dh in range(2):
                sl = slice(dh * 512, (dh + 1) * 512)
                nc.vector.tensor_add(yt[:, sl], ps_y[i][:, sl], x1[:, tb, sl])
            nc.sync.dma_start(out=yd[tb * P:(tb + 1) * P, :], in_=yt[:])


_NC_CACHE = {}


def _get_nc(replicas=1):
    if replicas not in _NC_CACHE:
        _NC_CACHE[replicas] = build_nc(replicas)
    return _NC_CACHE[replicas]


def _retile(w, kslices, mslices):
    """[K, M] -> [mslices, 128, kslices, 128], blk[m,p,k,c] = w[k*128+p, m*128+c]."""
    K, M = w.shape
    assert K == kslices * P and M == mslices * P
    return np.ascontiguousarray(
        w.reshape(kslices, P, mslices, P).transpose(2, 1, 0, 3))


def make_in_maps(x, attention_mask, wq, bq, wk, bk, wv, bv, wo, bo,
                 ln1_s, ln1_b, ln2_s, ln2_b, w1, b1, w2, b2):
    f = lambda a: np.asarray(a, dtype=np.float32)
    cb = lambda a: np.ascontiguousarray(a.astype(NPBF))
    wq, wk, wv, wo, w1, w2 = f(wq), f(wk), f(wv), f(wo), f(w1), f(w2)
    bq, bk, bv, bo, b1, b2 = f(bq), f(bk), f(bv), f(bo), f(b1), f(b2)
    s1, b1n, s2, b2n = f(ln1_s), f(ln1_b), f(ln2_s), f(ln2_b)
    # Fold LN affine into the consuming projections:
    #   (n*s + b) @ W + c == n @ (s[:,None]*W) + (b @ W + c)
    wq_f, bq_f = s1[:, None] * wq, b1n @ wq + bq
    wk_f, bk_f = s1[:, None] * wk, b1n @ wk + bk
    wv_f, bv_f = s1[:, None] * wv, b1n @ wv + bv
    w1_f, b1_f = s2[:, None] * w1, b2n @ w1 + b1
    shared = {
        "wqr": cb(_retile(wq_f, FC, FC)),
        "wkr": cb(_retile(wk_f, FC, FC)),
        "wvr": cb(wv_f.reshape(FC, P, D)),
        "wor": cb(wo.reshape(FC, P, D)),
        "w1r": cb(_retile(w1_f, FC, MC)),
        "w2r": cb(w2.reshape(MC, P, D)),
        "bq": bq_f, "bk": bk_f, "bvd": bv_f, "bod": bo,
        "b1": b1_f, "b2d": b2,
    }
    x = f(x)
    m = np.asarray(attention_mask, dtype=np.int32)
    return [dict(shared, x=np.ascontiguousarray(x[c]),
                 mask=np.ascontiguousarray(m[c])) for c in range(NCORES)]


def kernel(**inputs):
    from concourse.bass_utils import run_bass_kernel_spmd

    nc = _get_nc()
    in_maps = make_in_maps(**inputs)
    res = run_bass_kernel_spmd(nc, in_maps, core_ids=list(range(NCORES)))
    out = np.stack([res.results[c]["y"] for c in range(NCORES)], axis=0)
    return out.astype(np.float32)
